# revision 25
# baseline (speedup 1.0000x reference)
"""Trainium2 Bass kernel for nn_CutLayer (histogram_binning).

Strategy (data-parallel over events, 8 cores):
  L1: per-core min/max of the feature column (device reduce).
  L2: per-core exact per-edge cumulative counts, split across two engines:
      - Vector (DVE): scalar_tensor_tensor (x <= e) * w with w = 1 + 4096*y,
        per-partition accumulated over 3906-element halves, packing
        count and signal-count into one exact fp32 integer.
      - Scalar (ACT): sign(x - e) with per-partition accumulation on both
        the full stream and a signal-masked stream; counts recovered as
        (N + ties - sum_sign) / 2 (exact +-1 sums).
  host: combine counts, repair lt/le tie counts from a tiny candidate set,
      replicate the reference's tiny E^2 pair search bit-exactly with
      eager CPU jax, producing (lower, upper, case).
  L3: per-core case-specialized predicate (4 lazily-built programs; only
      the dispatched case compiles): cases 0/1 are a single 2x-rate
      tensor_scalar compare; cases 2/3 are one compare plus one fused
      scalar_tensor_tensor combine. All compares exact.

Events per core: 1_000_000; the device handles 128*7812 = 999_936 of them
(SBUF tile [128, 7812]); the 64-per-core remainder is handled exactly on
the host (512 events total).
"""

from contextlib import ExitStack

import numpy as np

import concourse.bass as bass
import concourse.mybir as mybir
from concourse.bass_utils import run_bass_kernel_spmd

N = 8_000_000
N_CORES = 8
CORE_N = N // N_CORES            # 1_000_000
P = 128
F = 7812                         # free-dim columns per partition
H = F // 2                       # packed-accum half (counts < 4096)
DEV_N = P * F                    # 999_936 device events per core
N_DEV_TOT = DEV_N * N_CORES      # 7_999_488
N_BINS = 50
E = N_BINS + 1                   # 51 edges
EPS = 1e-7
KD = 32                          # edges handled by the vector engine
KA = E - KD                      # edges handled by the scalar engine
PACK = 4096.0                    # signal-count multiplier (exact < 2^24)

FP32 = mybir.dt.float32
BF16 = mybir.dt.bfloat16
I32 = mybir.dt.int32
AX = mybir.AxisListType
OP = mybir.AluOpType
ACT = mybir.ActivationFunctionType

CORE_IDS = list(range(N_CORES))


# --------------------------------------------------------------------------
# Bass programs (built once per process)
# --------------------------------------------------------------------------

def _build_minmax():
    nc = bass.Bass()
    x = nc.declare_dram_parameter("x", [DEV_N], FP32, isOutput=False)
    mn = nc.declare_dram_parameter("mn", [P], FP32, isOutput=True)
    mx = nc.declare_dram_parameter("mx", [P], FP32, isOutput=True)
    with (
        nc.sbuf_tensor([P, F], FP32) as xt,
        nc.sbuf_tensor([P, 2], FP32) as acc,
        nc.semaphore() as dsem,
        nc.semaphore() as csem,
        nc.Block() as block,
    ):
        @block.sync
        def _(sync):
            sync.dma_start(xt[:], x[:].rearrange("(p f) -> p f", p=P)).then_inc(
                dsem, 16
            )
            sync.wait_ge(csem, 2)
            sync.dma_start(mn[:], acc[:, 0:1]).then_inc(dsem, 16)
            sync.dma_start(mx[:], acc[:, 1:2]).then_inc(dsem, 16)
            sync.wait_ge(dsem, 48)

        @block.vector
        def _(vector):
            vector.wait_ge(dsem, 16)
            vector.tensor_reduce(acc[:, 0:1], xt[:], axis=AX.X, op=OP.min).then_inc(
                csem, 1
            )
            vector.tensor_reduce(acc[:, 1:2], xt[:], axis=AX.X, op=OP.max).then_inc(
                csem, 1
            )
    return nc


def _build_counts():
    nc = bass.Bass()
    x = nc.declare_dram_parameter("x", [DEV_N], FP32, isOutput=False)
    w = nc.declare_dram_parameter("w", [DEV_N], FP32, isOutput=False)
    xs = nc.declare_dram_parameter("xs", [DEV_N], FP32, isOutput=False)
    ed = nc.declare_dram_parameter("edges", [P, 2 * E], FP32, isOutput=False)
    opk = nc.declare_dram_parameter("acc_pk", [P, 2 * KD], FP32, isOutput=True)
    osa = nc.declare_dram_parameter("acc_sa", [P, KA], FP32, isOutput=True)
    oss = nc.declare_dram_parameter("acc_ss", [P, KA], FP32, isOutput=True)
    with ExitStack() as es:
        ec = es.enter_context
        xt = ec(nc.sbuf_tensor([P, F], FP32))
        wt = ec(nc.sbuf_tensor([P, F], FP32))
        xst = ec(nc.sbuf_tensor([P, F], FP32))
        scr = ec(nc.sbuf_tensor([P, F], FP32))
        asca = ec(nc.sbuf_tensor([P, F], BF16))
        ascb = ec(nc.sbuf_tensor([P, F], BF16))
        edt = ec(nc.sbuf_tensor([P, 2 * E], FP32))
        apk = ec(nc.sbuf_tensor([P, 2 * KD], FP32))
        asa = ec(nc.sbuf_tensor([P, KA], FP32))
        ass = ec(nc.sbuf_tensor([P, KA], FP32))
        names = ["dse", "dsx", "dsx1", "dsw", "dsw1", "dss", "dso",
                 "csem", "v0", "v1", "t0", "t1"]
        dse, dsx, dsx1, dsw, dsw1, dss, dso, csem, v0, v1, t0, t1 = (
            ec(nc.semaphore(n)) for n in names
        )
        block = ec(nc.Block())
        @block.sync
        def _(sync):
            xv = x[:].rearrange("(p f) -> p f", p=P)
            wv = w[:].rearrange("(p f) -> p f", p=P)
            sync.dma_start(edt[:], ed[:]).then_inc(dse, 16)
            # interleave x/w halves so the vector engine can start on the
            # first data half as soon as possible
            sync.dma_start(xt[:, 0:H], xv[:, 0:H]).then_inc(dsx, 16)
            sync.dma_start(wt[:, 0:H], wv[:, 0:H]).then_inc(dsw, 16)
            sync.dma_start(xt[:, H:F], xv[:, H:F]).then_inc(dsx1, 16)
            sync.dma_start(wt[:, H:F], wv[:, H:F]).then_inc(dsw1, 16)
            sync.dma_start(xst[:], xs[:].rearrange("(p f) -> p f", p=P)).then_inc(
                dss, 16
            )
            # ACT typically retires first: ship its accumulators while the
            # vector engine finishes, then the packed accumulators.
            sync.wait_ge(t0, 2 * ((KA + 1) // 2))
            sync.wait_ge(t1, 2 * (KA // 2))
            sync.dma_start(osa[:], asa[:]).then_inc(dso, 16)
            sync.dma_start(oss[:], ass[:]).then_inc(dso, 16)
            sync.wait_ge(v0, KD)
            sync.wait_ge(v1, KD)
            sync.dma_start(opk[:], apk[:]).then_inc(dso, 16)
            sync.wait_ge(dso, 48)

        @block.vector
        def _(vector):
            # phase 1: all edges on data half 0 (needs edges + x0 + w0),
            # phase 2: all edges on data half 1 — accumulator slots are
            # per (edge, half) so order is free. Scratch regions ping-pong
            # by instruction parity with retirement semaphores.
            vector.wait_ge(dse, 16)
            vector.wait_ge(dsx, 16)
            vector.wait_ge(dsw, 16)
            nh = [0, 0]  # completed instruction count per scratch region
            ninstr = 0
            for dh in range(2):
                hs = dh * H
                if dh == 1:
                    vector.wait_ge(dsx1, 16)
                    vector.wait_ge(dsw1, 16)
                for i in range(KD):
                    e = edt[:, i : i + 1]
                    rr = ninstr % 2
                    if nh[rr] >= 1:
                        vector.wait_ge([v0, v1][rr], nh[rr])
                    vector.scalar_tensor_tensor(
                        scr[:, rr * H : rr * H + H],
                        xt[:, hs : hs + H],
                        e,
                        wt[:, hs : hs + H],
                        op0=OP.is_le,
                        op1=OP.mult,
                        accum_out=apk[:, 2 * i + dh : 2 * i + dh + 1],
                    ).then_inc([v0, v1][rr], 1)
                    nh[rr] += 1
                    ninstr += 1

        @block.scalar
        def _(scalar):
            # x-stream first (needs edges + x only)
            scalar.wait_ge(dse, 16)
            scalar.wait_ge(dsx, 16)
            scalar.wait_ge(dsx1, 16)
            na = [0, 0]
            scrs = [asca, ascb]
            sems = [t0, t1]
            for i in range(KA):
                ne = edt[:, E + KD + i : E + KD + i + 1]  # negated edge
                hh = i % 2
                if na[hh] >= 1:
                    scalar.wait_ge(sems[hh], na[hh])
                scalar.activation(
                    scrs[hh][:], xt[:], ACT.Sign, bias=ne, scale=1.0,
                    accum_out=asa[:, i : i + 1],
                ).then_inc(sems[hh], 1)
                na[hh] += 1
            # signal stream (needs xs)
            scalar.wait_ge(dss, 16)
            for i in range(KA):
                ne = edt[:, E + KD + i : E + KD + i + 1]
                hh = i % 2
                if na[hh] >= 1:
                    scalar.wait_ge(sems[hh], na[hh])
                scalar.activation(
                    scrs[hh][:], xst[:], ACT.Sign, bias=ne, scale=1.0,
                    accum_out=ass[:, i : i + 1],
                ).then_inc(sems[hh], 1)
                na[hh] += 1
    return nc


def _build_pred(case: int):
    """Case-specialized predicate:
    0: x <= lo                    (1 pass)
    1: x >= lo                    (1 pass)
    2: (x >= lo) & (x <= up)      (2 passes)
    3: (x <= lo) | (x >= up)      (2 passes, disjoint -> add)
    """
    nc = bass.Bass()
    x = nc.declare_dram_parameter("x", [DEV_N], FP32, isOutput=False)
    pr = nc.declare_dram_parameter("prm", [P, 8], FP32, isOutput=False)
    out = nc.declare_dram_parameter("pred", [DEV_N], I32, isOutput=True)
    with (
        nc.sbuf_tensor([P, F], FP32) as xt,
        nc.sbuf_tensor([P, F], FP32) as t,
        nc.sbuf_tensor([P, F], I32) as pi,
        nc.sbuf_tensor([P, 8], FP32) as prm,
        nc.semaphore() as dsem,
        nc.semaphore() as csem,
        nc.semaphore() as tsem,
        nc.Block() as block,
    ):
        @block.sync
        def _(sync):
            sync.dma_start(prm[:], pr[:]).then_inc(dsem, 16)
            sync.dma_start(xt[:], x[:].rearrange("(p f) -> p f", p=P)).then_inc(
                dsem, 16
            )
            sync.wait_ge(csem, 1)
            sync.dma_start(
                out[:].rearrange("(p f) -> p f", p=P), pi[:]
            ).then_inc(dsem, 16)
            sync.wait_ge(dsem, 48)

        @block.vector
        def _(vector):
            vector.wait_ge(dsem, 32)
            lo = prm[:, 0:1]
            up = prm[:, 1:2]
            if case == 0:
                vector.tensor_scalar(pi[:], xt[:], lo, None, OP.is_le).then_inc(
                    csem, 1
                )
            elif case == 1:
                vector.tensor_scalar(pi[:], xt[:], lo, None, OP.is_ge).then_inc(
                    csem, 1
                )
            elif case == 2:
                vector.tensor_scalar(t[:], xt[:], up, None, OP.is_le).then_inc(
                    tsem, 1
                )
                vector.wait_ge(tsem, 1)
                vector.scalar_tensor_tensor(
                    pi[:], xt[:], lo, t[:], op0=OP.is_ge, op1=OP.mult
                ).then_inc(csem, 1)
            else:
                vector.tensor_scalar(t[:], xt[:], up, None, OP.is_ge).then_inc(
                    tsem, 1
                )
                vector.wait_ge(tsem, 1)
                vector.scalar_tensor_tensor(
                    pi[:], xt[:], lo, t[:], op0=OP.is_le, op1=OP.add
                ).then_inc(csem, 1)
    return nc


_PROGRAMS: dict = {}


def _prog(name):
    if name not in _PROGRAMS:
        if name.startswith("pred"):
            _PROGRAMS[name] = _build_pred(int(name[4:]))
        else:
            _PROGRAMS[name] = {
                "minmax": _build_minmax,
                "counts": _build_counts,
            }[name]()
    return _PROGRAMS[name]


# --------------------------------------------------------------------------
# Host orchestration
# --------------------------------------------------------------------------

LAST_EXEC_NS: list = []


_CACHE_SET = False


def _enable_jit_cache():
    # Persist compiled executables (which embed the NEFF) across processes;
    # harmless no-op if the backend doesn't support serialization.
    global _CACHE_SET
    if _CACHE_SET:
        return
    _CACHE_SET = True
    try:
        import jax

        jax.config.update("jax_compilation_cache_dir", "/tmp/jax_bass_cache")
        jax.config.update("jax_persistent_cache_min_compile_time_secs", 1.0)
        jax.config.update("jax_persistent_cache_min_entry_size_bytes", 0)
    except Exception:
        pass


def _run(name, in_maps):
    import os

    _enable_jit_cache()
    trace = bool(int(os.environ.get("BASS_KERNEL_PROFILE", "0")))
    r = run_bass_kernel_spmd(_prog(name), in_maps, CORE_IDS, trace=trace)
    if trace:
        LAST_EXEC_NS.append((name, r.exec_time_ns, r.mean_exec_time_ns))
    return r.results


def _dev_shard(arr, c):
    return arr[c * CORE_N : c * CORE_N + DEV_N]


def _tail_shard(arr, c):
    return arr[c * CORE_N + DEV_N : (c + 1) * CORE_N]


def kernel(inputs: np.ndarray, targets: np.ndarray) -> np.ndarray:
    x_full = np.ascontiguousarray(inputs[:, 0]).astype(np.float32, copy=False)
    y_full = np.asarray(targets)

    tails_x = [_tail_shard(x_full, c) for c in CORE_IDS]
    tails_y = [_tail_shard(y_full, c) for c in CORE_IDS]
    tail_x = np.concatenate(tails_x)
    tail_y = np.concatenate(tails_y)

    # ---- L1: global min/max -------------------------------------------------
    LAST_EXEC_NS.clear()
    res1 = _run("minmax", [{"x": _dev_shard(x_full, c)} for c in CORE_IDS])
    gmin = np.float32(min(min(r["mn"].min() for r in res1), tail_x.min()))
    gmax = np.float32(max(max(r["mx"].max() for r in res1), tail_x.max()))

    # ---- edges: replicate jnp.linspace bit-exactly (eager CPU jax) ----------
    import jax
    import jax.numpy as jnp

    cpu = jax.devices("cpu")[0]
    with jax.default_device(cpu):
        edges = np.asarray(jnp.linspace(jnp.float32(gmin), jnp.float32(gmax), E))

    # ---- L2: per-edge counts ------------------------------------------------
    sig_mask = y_full == 1
    # Finite sentinel above every possible edge (sim paths reject inf inputs).
    sent = np.float32(np.finfo(np.float32).max)
    x_sig = np.where(sig_mask, x_full, sent).astype(np.float32)
    w_full = (1.0 + PACK * sig_mask).astype(np.float32)
    ed_in = np.concatenate([edges, -edges]).astype(np.float32)
    edges_rep = np.ascontiguousarray(np.broadcast_to(ed_in, (P, 2 * E)))

    res2 = _run(
        "counts",
        [
            {
                "x": _dev_shard(x_full, c),
                "w": _dev_shard(w_full, c),
                "xs": _dev_shard(x_sig, c),
                "edges": edges_rep,
            }
            for c in CORE_IDS
        ],
    )

    # ---- exact tie counts (x == edge) from a tiny candidate set -------------
    h = (np.float32(gmax) - np.float32(gmin)) / np.float32(N_BINS)
    inv_h = np.float32(1.0) / h if h != 0 else np.float32(0.0)
    u = (x_full - gmin) * inv_h
    r_near = np.rint(u)
    cand = np.abs(u - r_near) < np.float32(0.01)
    idx = np.flatnonzero(cand)
    T_all = np.zeros(E, np.float64)
    Tsig_all = np.zeros(E, np.float64)
    T_dev = np.zeros(E, np.float64)
    Tsig_dev = np.zeros(E, np.float64)
    if idx.size:
        kn = np.clip(r_near[idx].astype(np.int64), 0, E - 1)
        is_tie = x_full[idx] == edges[kn]
        tidx = idx[is_tie]
        tie_k = kn[is_tie]
        tie_sig = sig_mask[tidx]
        tie_dev = (tidx % CORE_N) < DEV_N
        np.add.at(T_all, tie_k, 1.0)
        np.add.at(Tsig_all, tie_k[tie_sig], 1.0)
        np.add.at(T_dev, tie_k[tie_dev], 1.0)
        np.add.at(Tsig_dev, tie_k[tie_dev & tie_sig], 1.0)

    # ---- decode device counts ----------------------------------------------
    cnt_le = np.zeros(E, np.float64)
    sig_le = np.zeros(E, np.float64)
    cnt_pk = np.zeros(2 * KD, np.int64)
    sig_pk = np.zeros(2 * KD, np.int64)
    sa = np.zeros(KA, np.float64)
    ss = np.zeros(KA, np.float64)
    for r in res2:
        # decode per accumulator slot (each packs cnt<4096 with 4096*sig)
        a = r["acc_pk"].astype(np.int64)
        s_part = a // int(PACK)
        c_part = a - int(PACK) * s_part
        cnt_pk += c_part.sum(axis=0)
        sig_pk += s_part.sum(axis=0)
        sa += r["acc_sa"].astype(np.float64).sum(axis=0)
        ss += r["acc_ss"].astype(np.float64).sum(axis=0)
    cnt_le[:KD] = cnt_pk.reshape(KD, 2).sum(axis=1)
    sig_le[:KD] = sig_pk.reshape(KD, 2).sum(axis=1)
    cnt_le[KD:] = (N_DEV_TOT + T_dev[KD:] - sa) / 2.0
    sig_le[KD:] = (N_DEV_TOT + Tsig_dev[KD:] - ss) / 2.0

    # tail events, exact
    cnt_le += (tail_x[:, None] <= edges[None, :]).sum(axis=0)
    sig_le += (tail_x[tail_y == 1][:, None] <= edges[None, :]).sum(axis=0)

    cnt_lt = cnt_le - T_all
    sig_lt = sig_le - Tsig_all

    ns_le = sig_le.astype(np.float32)
    ns_lt = sig_lt.astype(np.float32)
    nb_le = (cnt_le - sig_le).astype(np.float32)
    nb_lt = (cnt_lt - sig_lt).astype(np.float32)

    # ---- replicate the reference's tiny pair search (eager CPU jax) ---------
    with jax.default_device(cpu):
        ns_le_j = jnp.asarray(ns_le)
        ns_lt_j = jnp.asarray(ns_lt)
        nb_le_j = jnp.asarray(nb_le)
        nb_lt_j = jnp.asarray(nb_lt)
        n_f = jnp.float32(N)
        Ns = ns_le_j[-1]
        Nb = n_f - Ns

        hist0 = nb_le_j[1:] - nb_lt_j[:-1]
        hist1 = ns_le_j[1:] - ns_lt_j[:-1]

        gt0 = hist0 > hist1
        cand0 = jnp.logical_xor(gt0[:-1], gt0[1:]) & (hist0[:-1] > 0)
        gt1 = hist1 > hist0
        cand1 = jnp.logical_xor(gt1[:-1], gt1[1:]) & (hist1[:-1] > 0)
        mask = jnp.zeros((E,), bool).at[1:N_BINS].set(cand0 | cand1)
        cnt = jnp.sum(mask)
        mask = mask.at[-1].set(mask[-1] | (cnt == 1))

        a_c = -jnp.log1p(jnp.float32(-EPS))
        b_c = -jnp.log(jnp.float32(EPS))

        def bce(correct):
            return ((n_f - correct) * b_c + correct * a_c) / n_f

        c0 = ns_le_j + (Nb - nb_le_j)
        c1 = (Ns - ns_lt_j) + nb_lt_j
        c2 = (ns_le_j[None, :] - ns_lt_j[:, None]) + Nb - (
            nb_le_j[None, :] - nb_lt_j[:, None]
        )
        c3 = ns_le_j[:, None] + (Ns - ns_lt_j[None, :]) + (
            nb_le_j[None, :] - nb_lt_j[:, None]
        )

        L = jnp.stack(
            [
                jnp.broadcast_to(bce(c0)[:, None], (E, E)),
                jnp.broadcast_to(bce(c1)[:, None], (E, E)),
                bce(c2),
                bce(c3),
            ]
        )
        per_pair_min = jnp.min(L, axis=0)
        per_pair_case = jnp.argmin(L, axis=0)

        idxs = jnp.arange(E)
        valid = mask[:, None] & mask[None, :] & (idxs[:, None] < idxs[None, :])
        flat = jnp.argmin(jnp.where(valid, per_pair_min, jnp.inf))
        i = int(flat) // E
        j = int(flat) % E
        lower = np.float32(edges[i])
        upper = np.float32(edges[j])
        case = int(per_pair_case[i, j])

    # ---- L3: predicate (case-specialized program; exact compares) --------
    prm = np.zeros((P, 8), np.float32)
    prm[:, 0] = lower
    prm[:, 1] = upper

    res3 = _run(
        f"pred{case}", [{"x": _dev_shard(x_full, c), "prm": prm} for c in CORE_IDS]
    )

    out = np.empty(N, np.int32)
    for c in CORE_IDS:
        out[c * CORE_N : c * CORE_N + DEV_N] = res3[c]["pred"]
        tx = tails_x[c]
        if case == 0:
            tp = tx <= lower
        elif case == 1:
            tp = tx >= lower
        elif case == 2:
            tp = (tx >= lower) & (tx <= upper)
        else:
            tp = (tx <= lower) | (tx >= upper)
        out[c * CORE_N + DEV_N : (c + 1) * CORE_N] = tp.astype(np.int32)
    return out
